# revision 10
# baseline (speedup 1.0000x reference)
"""Trainium2 Bass kernel for the NoisyTopK MoE layer (B=2,T=2048,D=1024,H=4096,O=1024,E=8,K=2).

Strategy (expert-parallel, 8 cores = 8 experts):
  * Host: compute top-2 routing indices (tiny numpy), gather each expert's
    tokens, pad to a common capacity C (multiple of 128).
  * Device (per core, SPMD — same program, per-expert data):
      - routing: noisy logits -> top-2 -> sparse softmax gates, on-chip,
        for the gathered tokens (replicated gate params)
      - expert FFN: out = (relu(x @ W1 + b1) @ W2 + b2) * gate_e
        fused MM1->MM2 per H-slice, fp32r matmuls, W2 resident in SBUF,
        W1 streamed per chunk of 384 tokens.
  * Host: scatter-add the per-expert outputs back to [B,T,O]
    (equivalent to the all-reduce of the gated combine).
"""

import os
import time

import numpy as np

P = 128
B, T, D, H, O, E = 2, 2048, 1024, 4096, 1024, 8
KD = D // P   # 8  k-tiles over D
KH = H // P   # 32 k-tiles over H (= number of m-slices of MM1)
NM = H // P   # 32 m-slices
OS = 2        # O-slices of 512
TB_PER_CHUNK = 3  # 384 tokens per chunk

_NC_CACHE = {}
LAST_RUN = {}


def _build_nc(C):
    import concourse.mybir as mybir
    import concourse.tile as tile
    from concourse import bacc

    f32 = mybir.dt.float32
    f32r = mybir.dt.float32r
    AF = mybir.ActivationFunctionType
    ALU = mybir.AluOpType
    AX = mybir.AxisListType

    NTB = C // P
    chunks = []
    tb0 = 0
    while tb0 < NTB:
        n = min(TB_PER_CHUNK, NTB - tb0)
        chunks.append((tb0, n))
        tb0 += n

    # Bacc (not plain Bass): its compile() pass splits multi-wait matmuls
    # (HW allows a single sync-wait on the fused LDWEIGHTS+MATMULT).
    nc = bacc.Bacc()
    xT_d = nc.declare_dram_parameter("xT", [P, KD, C], f32, isOutput=False)
    w1_d = nc.declare_dram_parameter("w1s", [NM, P, KD, P], f32, isOutput=False)
    w2_d = nc.declare_dram_parameter("w2s", [P, KH, O], f32, isOutput=False)
    wgn_d = nc.declare_dram_parameter("wgn", [P, KD, 2 * E], f32, isOutput=False)
    bgbn_d = nc.declare_dram_parameter("bgbn", [1, 2 * E], f32, isOutput=False)
    b1_d = nc.declare_dram_parameter("b1s", [P, NM], f32, isOutput=False)
    b2_d = nc.declare_dram_parameter("b2e", [1, O], f32, isOutput=False)
    nz_d = nc.declare_dram_parameter("noise_g", [P, NTB, E], f32, isOutput=False)
    sel_d = nc.declare_dram_parameter("sel", [1, E], f32, isOutput=False)
    out_d = nc.declare_dram_parameter("out", [NTB, P, O], f32, isOutput=True)

    with tile.TileContext(nc) as tc:
        with (
            tc.tile_pool(name="singles", bufs=1) as singles,
            tc.tile_pool(name="w1pool", bufs=3) as w1pool,
            tc.tile_pool(name="xpool", bufs=2) as xpool,
            tc.tile_pool(name="hpool", bufs=3) as hpool,
            tc.tile_pool(name="rpool", bufs=2) as rpool,
            tc.tile_pool(name="gpool", bufs=2) as gpool,
            tc.tile_pool(name="spool", bufs=2) as spool,
            tc.tile_pool(name="psA", bufs=6, space="PSUM") as psA,
            tc.tile_pool(name="psB", bufs=2, space="PSUM") as psB,
        ):
            # ---- resident tensors ----
            # (f32r tiles are bit-identical to f32; the tag satisfies the
            # BIR verifier's "producer must be rounded to FP32r" rule)
            w2_sb = singles.tile([P, KH, O], f32r)
            for kh in range(KH):
                nc.sync.dma_start(w2_sb[:, kh, :], w2_d[:, kh, :].bitcast(f32r))
            wgn_sb = singles.tile([P, KD, 2 * E], f32)
            nc.sync.dma_start(wgn_sb[:], wgn_d[:])
            bgbn_sb = singles.tile([P, 2 * E], f32)
            nc.sync.dma_start(bgbn_sb[:], bgbn_d[0].partition_broadcast(P))
            b1_sb = singles.tile([P, NM], f32)
            nc.sync.dma_start(b1_sb[:], b1_d[:])
            b2_sb = singles.tile([P, O], f32)
            nc.sync.dma_start(b2_sb[:], b2_d[0].partition_broadcast(P))
            sel_sb = singles.tile([P, E], f32)
            nc.sync.dma_start(sel_sb[:], sel_d[0].partition_broadcast(P))
            nz_sb = singles.tile([P, NTB, E], f32)
            nc.sync.dma_start(nz_sb[:], nz_d[:])

            for tb0, ntb in chunks:
                nt = ntb * P
                xs = xpool.tile([P, KD, TB_PER_CHUNK * P], f32r, tag="xs")
                nc.sync.dma_start(
                    xs[:, :, :nt],
                    xT_d[:, :, tb0 * P : tb0 * P + nt].bitcast(f32r),
                )

                # ---- routing for this chunk's token blocks ----
                gate = gpool.tile([P, TB_PER_CHUNK], f32, tag="gate")
                for j in range(ntb):
                    tb = tb0 + j
                    # full-fp32 x tile for routing (top-2 selection needs
                    # more precision than fp32r)
                    xr = rpool.tile([P, KD, P], f32, tag="xr")
                    nc.sync.dma_start(
                        xr[:], xT_d[:, :, (tb0 + j) * P : (tb0 + j + 1) * P]
                    )
                    ps = psB.tile([P, TB_PER_CHUNK * P], f32, tag="mm1ps")
                    pg = ps[:, : 2 * E]
                    for ko in range(KD):
                        nc.tensor.matmul(
                            pg,
                            xr[:, ko, :],
                            wgn_sb[:, ko, :],
                            start=(ko == 0),
                            stop=(ko == KD - 1),
                        )
                    nsb = rpool.tile([P, 2 * E], f32, tag="nsb")
                    nc.vector.tensor_add(nsb[:], pg, bgbn_sb[:])
                    # softplus = ln(1 + exp(z)); |z| <~ 5 here so exp is safe
                    spt = rpool.tile([P, E], f32, tag="spt")
                    nc.scalar.activation(spt[:], nsb[:, E:], AF.Exp)
                    nc.vector.tensor_scalar_add(spt[:], spt[:], 1.0)
                    nc.scalar.activation(spt[:], spt[:], AF.Ln)
                    noisy = rpool.tile([P, E], f32, tag="noisy")
                    nc.vector.tensor_mul(noisy[:], spt[:], nz_sb[:, tb, :])
                    nc.vector.tensor_add(noisy[:], noisy[:], nsb[:, :E])
                    m1 = rpool.tile([P, 1], f32, tag="m1")
                    nc.vector.reduce_max(m1[:], noisy[:], axis=AX.X)
                    eq = rpool.tile([P, E], f32, tag="eq")
                    nc.vector.tensor_scalar(
                        eq[:], noisy[:], m1[:], None, ALU.is_ge
                    )
                    nc.vector.tensor_scalar_mul(eq[:], eq[:], -1e30)
                    nc.vector.tensor_add(eq[:], eq[:], noisy[:])
                    m2 = rpool.tile([P, 1], f32, tag="m2")
                    nc.vector.reduce_max(m2[:], eq[:], axis=AX.X)
                    m1n = rpool.tile([P, 1], f32, tag="m1n")
                    nc.vector.tensor_scalar_mul(m1n[:], m1[:], -1.0)
                    ex = rpool.tile([P, E], f32, tag="ex")
                    nc.scalar.activation(ex[:], noisy[:], AF.Exp, bias=m1n[:])
                    r2 = rpool.tile([P, 1], f32, tag="r2")
                    nc.scalar.activation(r2[:], m2[:], AF.Exp, bias=m1n[:])
                    nc.vector.tensor_scalar_add(r2[:], r2[:], 1.0)
                    rden = rpool.tile([P, 1], f32, tag="rden")
                    nc.vector.reciprocal(rden[:], r2[:])
                    msk = rpool.tile([P, E], f32, tag="msk")
                    nc.vector.tensor_scalar(
                        msk[:], noisy[:], m2[:], None, ALU.is_ge
                    )
                    nc.vector.tensor_mul(msk[:], msk[:], ex[:])
                    nc.vector.tensor_scalar_mul(msk[:], msk[:], rden[:])
                    nc.vector.tensor_mul(msk[:], msk[:], sel_sb[:])
                    nc.vector.reduce_sum(gate[:, j : j + 1], msk[:], axis=AX.X)

                # ---- fused FFN: MM1 (per H-slice) -> MM2 accumulate ----
                accs = [
                    [
                        psA.tile([P, 512], f32, tag="acc", name=f"acc_{j}_{osl}")
                        for osl in range(OS)
                    ]
                    for j in range(ntb)
                ]
                for m in range(NM):
                    w1t = w1pool.tile([P, KD, P], f32r, tag="w1t")
                    nc.sync.dma_start(w1t[:], w1_d[m].bitcast(f32r))
                    hps = psB.tile([P, TB_PER_CHUNK * P], f32, tag="mm1ps")
                    hw = hps[:, :nt]
                    for ko in range(KD):
                        nc.tensor.matmul(
                            hw,
                            w1t[:, ko, :],
                            xs[:, ko, :nt],
                            start=(ko == 0),
                            stop=(ko == KD - 1),
                        )
                    hm = hpool.tile([P, TB_PER_CHUNK * P], f32r, tag="hm")
                    nc.scalar.activation(
                        hm[:, :nt], hw, AF.Relu, bias=b1_sb[:, m : m + 1]
                    )
                    for j in range(ntb):
                        for osl in range(OS):
                            nc.tensor.matmul(
                                accs[j][osl],
                                hm[:, j * P : (j + 1) * P],
                                w2_sb[:, m, osl * 512 : (osl + 1) * 512],
                                start=(m == 0),
                                stop=(m == NM - 1),
                            )

                # ---- evict: (acc + b2) * gate -> DRAM ----
                for j in range(ntb):
                    st = spool.tile([P, O], f32, tag="st")
                    for osl in range(OS):
                        sl = slice(osl * 512, (osl + 1) * 512)
                        nc.vector.tensor_add(st[:, sl], accs[j][osl], b2_sb[:, sl])
                        nc.vector.tensor_scalar_mul(
                            st[:, sl], st[:, sl], gate[:, j : j + 1]
                        )
                    nc.sync.dma_start(out_d[tb0 + j], st[:])

    nc.finalize()
    return nc


def _routing_host(xf, nf, Wg, bg, Wn, bn):
    """Top-2 expert indices per token (selection only; gates are computed
    on device)."""
    logits = xf @ Wg + bg
    nl = xf @ Wn + bn
    sp = np.logaddexp(0.0, nl)
    noisy = logits + nf * sp
    order = np.argpartition(-noisy, 2, axis=1)[:, :2]
    mask = np.zeros(noisy.shape, dtype=bool)
    mask[np.arange(noisy.shape[0])[:, None], order] = True
    return mask


def _prep_core(xf, nf, idx, C, Wg, Wn, bg, bn, W1e, b1e, W2e, b2e, e):
    n = len(idx)
    x_g = np.zeros((C, D), np.float32)
    x_g[:n] = xf[idx]
    nz_g = np.zeros((C, E), np.float32)
    nz_g[:n] = nf[idx]
    NTB = C // P
    return {
        "xT": np.ascontiguousarray(x_g.reshape(C, KD, P).transpose(2, 1, 0)),
        "w1s": np.ascontiguousarray(
            W1e.reshape(KD, P, NM, P).transpose(2, 1, 0, 3)
        ),
        "w2s": np.ascontiguousarray(W2e.reshape(KH, P, O).transpose(1, 0, 2)),
        "wgn": np.ascontiguousarray(
            np.concatenate([Wg, Wn], axis=1).reshape(KD, P, 2 * E).transpose(1, 0, 2)
        ),
        "bgbn": np.concatenate([bg, bn])[None, :].astype(np.float32),
        "b1s": np.ascontiguousarray(b1e.reshape(NM, P).T),
        "b2e": b2e[None, :].astype(np.float32),
        "noise_g": np.ascontiguousarray(
            nz_g.reshape(NTB, P, E).transpose(1, 0, 2)
        ),
        "sel": np.eye(E, dtype=np.float32)[e][None, :],
    }


def kernel(x, noise, Wg, bg, Wn, bn, W1, b1, W2, b2):
    from concourse.bass_utils import run_bass_kernel_spmd

    x = np.asarray(x, np.float32)
    noise = np.asarray(noise, np.float32)
    Wg = np.asarray(Wg, np.float32)
    bg = np.asarray(bg, np.float32)
    Wn = np.asarray(Wn, np.float32)
    bn = np.asarray(bn, np.float32)
    W1 = np.asarray(W1, np.float32)
    b1 = np.asarray(b1, np.float32)
    W2 = np.asarray(W2, np.float32)
    b2 = np.asarray(b2, np.float32)

    Bx, Tx, _ = x.shape
    ntok = Bx * Tx
    xf = x.reshape(ntok, D)
    nf = noise.reshape(ntok, E)

    mask = _routing_host(xf, nf, Wg, bg, Wn, bn)
    idx = [np.nonzero(mask[:, e])[0] for e in range(E)]
    C = max(P, int(np.ceil(max(len(i) for i in idx) / P) * P))

    if C not in _NC_CACHE:
        _NC_CACHE[C] = _build_nc(C)
    nc = _NC_CACHE[C]

    in_maps = [
        _prep_core(
            xf, nf, idx[e], C, Wg, Wn, bg, bn, W1[e], b1[e], W2[e], b2[e], e
        )
        for e in range(E)
    ]

    trace = bool(os.environ.get("MOE_TRACE"))
    t0 = time.time()
    res = run_bass_kernel_spmd(
        nc, in_maps, list(range(E)), trace=trace
    )
    t1 = time.time()
    LAST_RUN.clear()
    LAST_RUN.update(
        wall_s=t1 - t0,
        exec_time_ns=res.exec_time_ns,
        trace=res.instructions_and_trace[1] if res.instructions_and_trace else None,
    )

    out = np.zeros((ntok, O), np.float32)
    for e in range(E):
        n = len(idx[e])
        y = res.results[e]["out"].reshape(C, O)
        out[idx[e]] += y[:n]
    return out.reshape(Bx, Tx, O)


# revision 16
# speedup vs baseline: 1.2326x; 1.2326x over previous
"""Trainium2 Bass kernel for the NoisyTopK MoE layer (B=2,T=2048,D=1024,H=4096,O=1024,E=8,K=2).

Strategy (expert-parallel, 8 cores = 8 experts):
  * Host: compute top-2 routing indices (tiny numpy), gather each expert's
    tokens, pad to a common capacity C (multiple of 128).
  * Device (per core, SPMD — same program, per-expert data):
      - routing: noisy logits -> top-2 -> sparse softmax gates, on-chip,
        for the gathered tokens (replicated gate params)
      - expert FFN: out = (relu(x @ W1 + b1) @ W2 + b2) * gate_e
        fused MM1->MM2 per H-slice, fp32r matmuls, W2 resident in SBUF,
        W1 streamed per chunk of 384 tokens.
  * Host: scatter-add the per-expert outputs back to [B,T,O]
    (equivalent to the all-reduce of the gated combine).
"""

import os
import time

import numpy as np

P = 128
B, T, D, H, O, E = 2, 2048, 1024, 4096, 1024, 8
KD = D // P   # 8  k-tiles over D
KH = H // P   # 32 k-tiles over H (= number of m-slices of MM1)
NM = H // P   # 32 m-slices
OS = 2        # O-slices of 512
TB_PER_CHUNK = 3  # 384 tokens per chunk

_NC_CACHE = {}
LAST_RUN = {}


def _build_nc(C):
    import concourse.mybir as mybir
    import concourse.tile as tile
    from concourse import bacc

    f32 = mybir.dt.float32
    f16 = mybir.dt.float16
    AF = mybir.ActivationFunctionType
    ALU = mybir.AluOpType
    AX = mybir.AxisListType

    NTB = C // P
    chunks = []
    tb0 = 0
    while tb0 < NTB:
        n = min(TB_PER_CHUNK, NTB - tb0)
        chunks.append((tb0, n))
        tb0 += n

    # Bacc (not plain Bass): its compile() pass splits multi-wait matmuls
    # (HW allows a single sync-wait on the fused LDWEIGHTS+MATMULT).
    nc = bacc.Bacc()
    xT_d = nc.declare_dram_parameter("xT", [P, KD, C], f32, isOutput=False)
    xh_d = nc.declare_dram_parameter("xh", [P, KD, C], f16, isOutput=False)
    w1_d = nc.declare_dram_parameter("w1s", [NM, P, KD, P], f16, isOutput=False)
    w2_d = nc.declare_dram_parameter("w2s", [P, KH, O], f16, isOutput=False)
    wgn_d = nc.declare_dram_parameter("wgn", [P, KD, 2 * E], f32, isOutput=False)
    bgbn_d = nc.declare_dram_parameter("bgbn", [1, 2 * E], f32, isOutput=False)
    b1_d = nc.declare_dram_parameter("b1s", [P, NM], f32, isOutput=False)
    b2_d = nc.declare_dram_parameter("b2e", [1, O], f32, isOutput=False)
    nz_d = nc.declare_dram_parameter("noise_g", [P, NTB, E], f32, isOutput=False)
    sel_d = nc.declare_dram_parameter("sel", [1, E], f32, isOutput=False)
    out_d = nc.declare_dram_parameter("out", [NTB, P, O], f32, isOutput=True)

    with tile.TileContext(nc) as tc:
        with (
            tc.tile_pool(name="singles", bufs=1) as singles,
            tc.tile_pool(name="w1pool", bufs=3) as w1pool,
            tc.tile_pool(name="xpool", bufs=2) as xpool,
            tc.tile_pool(name="hpool", bufs=3) as hpool,
            tc.tile_pool(name="rpool", bufs=2) as rpool,
            tc.tile_pool(name="gpool", bufs=2) as gpool,
            tc.tile_pool(name="spool", bufs=2) as spool,
            tc.tile_pool(name="psA", bufs=6, space="PSUM") as psA,
            tc.tile_pool(name="psB", bufs=2, space="PSUM") as psB,
        ):
            # ---- resident tensors ----
            # W2 stays resident in SBUF; its slices are loaded just-in-time
            # inside chunk 0's m-loop so the first matmuls don't stall on a
            # bulk 8MB load.
            w2_sb = singles.tile([P, KH, O], f16)
            wgn_sb = singles.tile([P, KD, 2 * E], f32)
            nc.sync.dma_start(wgn_sb[:], wgn_d[:])
            bgbn_sb = singles.tile([P, 2 * E], f32)
            nc.sync.dma_start(bgbn_sb[:], bgbn_d[0].partition_broadcast(P))
            b1_sb = singles.tile([P, NM], f32)
            nc.sync.dma_start(b1_sb[:], b1_d[:])
            b2_sb = singles.tile([P, O], f32)
            nc.sync.dma_start(b2_sb[:], b2_d[0].partition_broadcast(P))
            sel_sb = singles.tile([P, E], f32)
            nc.sync.dma_start(sel_sb[:], sel_d[0].partition_broadcast(P))
            nz_sb = singles.tile([P, NTB, E], f32)
            nc.sync.dma_start(nz_sb[:], nz_d[:])

            for ci, (tb0, ntb) in enumerate(chunks):
                nt = ntb * P
                xs = xpool.tile([P, KD, TB_PER_CHUNK * P], f16, tag="xs")
                nc.sync.dma_start(
                    xs[:, :, :nt],
                    xh_d[:, :, tb0 * P : tb0 * P + nt],
                )

                # ---- routing for this chunk's token blocks ----
                gate = gpool.tile([P, TB_PER_CHUNK], f32, tag="gate")
                for j in range(ntb):
                    tb = tb0 + j
                    # full-fp32 x tile for routing (top-2 selection needs
                    # more precision than fp32r)
                    xr = rpool.tile([P, KD, P], f32, tag="xr")
                    nc.sync.dma_start(
                        xr[:], xT_d[:, :, (tb0 + j) * P : (tb0 + j + 1) * P]
                    )
                    ps = psB.tile([P, TB_PER_CHUNK * P], f32, tag="mm1ps")
                    pg = ps[:, : 2 * E]
                    for ko in range(KD):
                        nc.tensor.matmul(
                            pg,
                            xr[:, ko, :],
                            wgn_sb[:, ko, :],
                            start=(ko == 0),
                            stop=(ko == KD - 1),
                        )
                    nsb = rpool.tile([P, 2 * E], f32, tag="nsb")
                    nc.vector.tensor_add(nsb[:], pg, bgbn_sb[:])
                    # softplus = ln(1 + exp(z)); |z| <~ 5 here so exp is safe
                    spt = rpool.tile([P, E], f32, tag="spt")
                    nc.scalar.activation(spt[:], nsb[:, E:], AF.Exp)
                    nc.vector.tensor_scalar_add(spt[:], spt[:], 1.0)
                    nc.scalar.activation(spt[:], spt[:], AF.Ln)
                    noisy = rpool.tile([P, E], f32, tag="noisy")
                    nc.vector.tensor_mul(noisy[:], spt[:], nz_sb[:, tb, :])
                    nc.vector.tensor_add(noisy[:], noisy[:], nsb[:, :E])
                    m1 = rpool.tile([P, 1], f32, tag="m1")
                    nc.vector.reduce_max(m1[:], noisy[:], axis=AX.X)
                    eq = rpool.tile([P, E], f32, tag="eq")
                    nc.vector.tensor_scalar(
                        eq[:], noisy[:], m1[:], None, ALU.is_ge
                    )
                    nc.vector.tensor_scalar_mul(eq[:], eq[:], -1e30)
                    nc.vector.tensor_add(eq[:], eq[:], noisy[:])
                    m2 = rpool.tile([P, 1], f32, tag="m2")
                    nc.vector.reduce_max(m2[:], eq[:], axis=AX.X)
                    m1n = rpool.tile([P, 1], f32, tag="m1n")
                    nc.vector.tensor_scalar_mul(m1n[:], m1[:], -1.0)
                    ex = rpool.tile([P, E], f32, tag="ex")
                    nc.scalar.activation(ex[:], noisy[:], AF.Exp, bias=m1n[:])
                    r2 = rpool.tile([P, 1], f32, tag="r2")
                    nc.scalar.activation(r2[:], m2[:], AF.Exp, bias=m1n[:])
                    nc.vector.tensor_scalar_add(r2[:], r2[:], 1.0)
                    rden = rpool.tile([P, 1], f32, tag="rden")
                    nc.vector.reciprocal(rden[:], r2[:])
                    msk = rpool.tile([P, E], f32, tag="msk")
                    nc.vector.tensor_scalar(
                        msk[:], noisy[:], m2[:], None, ALU.is_ge
                    )
                    nc.vector.tensor_mul(msk[:], msk[:], ex[:])
                    nc.vector.tensor_scalar_mul(msk[:], msk[:], rden[:])
                    nc.vector.tensor_mul(msk[:], msk[:], sel_sb[:])
                    nc.vector.reduce_sum(gate[:, j : j + 1], msk[:], axis=AX.X)

                # ---- fused FFN: MM1 (per H-slice) -> MM2 accumulate ----
                accs = [
                    [
                        psA.tile([P, 512], f32, tag="acc", name=f"acc_{j}_{osl}")
                        for osl in range(OS)
                    ]
                    for j in range(ntb)
                ]
                for m in range(NM):
                    w1t = w1pool.tile([P, KD, P], f16, tag="w1t")
                    nc.sync.dma_start(w1t[:], w1_d[m])
                    if ci == 0:
                        nc.sync.dma_start(w2_sb[:, m, :], w2_d[:, m, :])
                    hps = psB.tile([P, TB_PER_CHUNK * P], f32, tag="mm1ps")
                    hw = hps[:, :nt]
                    for ko in range(KD):
                        nc.tensor.matmul(
                            hw,
                            w1t[:, ko, :],
                            xs[:, ko, :nt],
                            start=(ko == 0),
                            stop=(ko == KD - 1),
                        )
                    hm = hpool.tile([P, TB_PER_CHUNK * P], f16, tag="hm")
                    nc.scalar.activation(
                        hm[:, :nt], hw, AF.Relu, bias=b1_sb[:, m : m + 1]
                    )
                    for j in range(ntb):
                        for osl in range(OS):
                            nc.tensor.matmul(
                                accs[j][osl],
                                hm[:, j * P : (j + 1) * P],
                                w2_sb[:, m, osl * 512 : (osl + 1) * 512],
                                start=(m == 0),
                                stop=(m == NM - 1),
                            )

                # ---- evict: (acc + b2) * gate -> DRAM ----
                for j in range(ntb):
                    st = spool.tile([P, O], f32, tag="st")
                    for osl in range(OS):
                        sl = slice(osl * 512, (osl + 1) * 512)
                        nc.vector.tensor_add(st[:, sl], accs[j][osl], b2_sb[:, sl])
                        nc.vector.tensor_scalar_mul(
                            st[:, sl], st[:, sl], gate[:, j : j + 1]
                        )
                    nc.sync.dma_start(out_d[tb0 + j], st[:])

    nc.finalize()
    return nc


def _routing_host(xf, nf, Wg, bg, Wn, bn):
    """Top-2 expert indices per token (selection only; gates are computed
    on device)."""
    logits = xf @ Wg + bg
    nl = xf @ Wn + bn
    sp = np.logaddexp(0.0, nl)
    noisy = logits + nf * sp
    order = np.argpartition(-noisy, 2, axis=1)[:, :2]
    mask = np.zeros(noisy.shape, dtype=bool)
    mask[np.arange(noisy.shape[0])[:, None], order] = True
    return mask


def _prep_core(xf, nf, idx, C, Wg, Wn, bg, bn, W1e, b1e, W2e, b2e, e):
    n = len(idx)
    x_g = np.zeros((C, D), np.float32)
    x_g[:n] = xf[idx]
    nz_g = np.zeros((C, E), np.float32)
    nz_g[:n] = nf[idx]
    NTB = C // P
    xT = np.ascontiguousarray(x_g.reshape(C, KD, P).transpose(2, 1, 0))
    return {
        "xT": xT,
        "xh": xT.astype(np.float16),
        "w1s": np.ascontiguousarray(
            W1e.reshape(KD, P, NM, P).transpose(2, 1, 0, 3)
        ).astype(np.float16),
        "w2s": np.ascontiguousarray(
            W2e.reshape(KH, P, O).transpose(1, 0, 2)
        ).astype(np.float16),
        "wgn": np.ascontiguousarray(
            np.concatenate([Wg, Wn], axis=1).reshape(KD, P, 2 * E).transpose(1, 0, 2)
        ),
        "bgbn": np.concatenate([bg, bn])[None, :].astype(np.float32),
        "b1s": np.ascontiguousarray(b1e.reshape(NM, P).T),
        "b2e": b2e[None, :].astype(np.float32),
        "noise_g": np.ascontiguousarray(
            nz_g.reshape(NTB, P, E).transpose(1, 0, 2)
        ),
        "sel": np.eye(E, dtype=np.float32)[e][None, :],
    }


def kernel(x, noise, Wg, bg, Wn, bn, W1, b1, W2, b2):
    from concourse.bass_utils import run_bass_kernel_spmd

    x = np.asarray(x, np.float32)
    noise = np.asarray(noise, np.float32)
    Wg = np.asarray(Wg, np.float32)
    bg = np.asarray(bg, np.float32)
    Wn = np.asarray(Wn, np.float32)
    bn = np.asarray(bn, np.float32)
    W1 = np.asarray(W1, np.float32)
    b1 = np.asarray(b1, np.float32)
    W2 = np.asarray(W2, np.float32)
    b2 = np.asarray(b2, np.float32)

    Bx, Tx, _ = x.shape
    ntok = Bx * Tx
    xf = x.reshape(ntok, D)
    nf = noise.reshape(ntok, E)

    mask = _routing_host(xf, nf, Wg, bg, Wn, bn)
    idx = [np.nonzero(mask[:, e])[0] for e in range(E)]
    C = max(P, int(np.ceil(max(len(i) for i in idx) / P) * P))

    if C not in _NC_CACHE:
        _NC_CACHE[C] = _build_nc(C)
    nc = _NC_CACHE[C]

    in_maps = [
        _prep_core(
            xf, nf, idx[e], C, Wg, Wn, bg, bn, W1[e], b1[e], W2[e], b2[e], e
        )
        for e in range(E)
    ]

    trace = bool(os.environ.get("MOE_TRACE"))
    t0 = time.time()
    res = run_bass_kernel_spmd(
        nc, in_maps, list(range(E)), trace=trace
    )
    t1 = time.time()
    LAST_RUN.clear()
    LAST_RUN.update(
        wall_s=t1 - t0,
        exec_time_ns=res.exec_time_ns,
        trace=res.instructions_and_trace[1] if res.instructions_and_trace else None,
    )

    out = np.zeros((ntok, O), np.float32)
    for e in range(E):
        n = len(idx[e])
        y = res.results[e]["out"].reshape(C, O)
        out[idx[e]] += y[:n]
    return out.reshape(Bx, Tx, O)


# revision 19
# speedup vs baseline: 1.2631x; 1.0248x over previous
"""Trainium2 Bass kernel for the NoisyTopK MoE layer (B=2,T=2048,D=1024,H=4096,O=1024,E=8,K=2).

Strategy (expert-parallel, 8 cores = 8 experts):
  * Host: compute top-2 routing indices (tiny numpy), gather each expert's
    tokens, pad to a common capacity C (multiple of 128).
  * Device (per core, SPMD — same program, per-expert data):
      - routing: noisy logits -> top-2 -> sparse softmax gates, on-chip,
        for the gathered tokens (replicated gate params)
      - expert FFN: out = (relu(x @ W1 + b1) @ W2 + b2) * gate_e
        fused MM1->MM2 per H-slice, fp32r matmuls, W2 resident in SBUF,
        W1 streamed per chunk of 384 tokens.
  * Host: scatter-add the per-expert outputs back to [B,T,O]
    (equivalent to the all-reduce of the gated combine).
"""

import os
import time

import numpy as np

P = 128
B, T, D, H, O, E = 2, 2048, 1024, 4096, 1024, 8
KD = D // P   # 8  k-tiles over D
KH = H // P   # 32 k-tiles over H (= number of m-slices of MM1)
NM = H // P   # 32 m-slices
OS = 2        # O-slices of 512
TB_PER_CHUNK = 3  # 384 tokens per chunk

_NC_CACHE = {}
LAST_RUN = {}


def _build_nc(C):
    import concourse.mybir as mybir
    import concourse.tile as tile
    from concourse import bacc

    f32 = mybir.dt.float32
    f16 = mybir.dt.float16
    AF = mybir.ActivationFunctionType
    ALU = mybir.AluOpType
    AX = mybir.AxisListType

    NTB = C // P
    chunks = []
    tb0 = 0
    while tb0 < NTB:
        n = min(TB_PER_CHUNK, NTB - tb0)
        chunks.append((tb0, n))
        tb0 += n

    # Bacc (not plain Bass): its compile() pass splits multi-wait matmuls
    # (HW allows a single sync-wait on the fused LDWEIGHTS+MATMULT).
    nc = bacc.Bacc()
    xT_d = nc.declare_dram_parameter("xT", [P, KD, C], f32, isOutput=False)
    xh_d = nc.declare_dram_parameter("xh", [P, KD, C], f16, isOutput=False)
    w1_d = nc.declare_dram_parameter("w1s", [NM, P, KD, P], f16, isOutput=False)
    w2_d = nc.declare_dram_parameter("w2s", [P, KH, O], f16, isOutput=False)
    wgn_d = nc.declare_dram_parameter("wgn", [P, KD, 2 * E], f32, isOutput=False)
    bgbn_d = nc.declare_dram_parameter("bgbn", [1, 2 * E], f32, isOutput=False)
    b1_d = nc.declare_dram_parameter("b1s", [P, NM], f32, isOutput=False)
    b2_d = nc.declare_dram_parameter("b2e", [1, O], f32, isOutput=False)
    nz_d = nc.declare_dram_parameter("noise_g", [P, NTB, E], f32, isOutput=False)
    sel_d = nc.declare_dram_parameter("sel", [1, E], f32, isOutput=False)
    out_d = nc.declare_dram_parameter("out", [NTB, P, O], f32, isOutput=True)

    with tile.TileContext(nc) as tc:
        with (
            tc.tile_pool(name="singles", bufs=1) as singles,
            tc.tile_pool(name="w1pool", bufs=3) as w1pool,
            tc.tile_pool(name="xpool", bufs=2) as xpool,
            tc.tile_pool(name="hpool", bufs=4) as hpool,
            tc.tile_pool(name="rpool", bufs=2) as rpool,
            tc.tile_pool(name="gpool", bufs=2) as gpool,
            tc.tile_pool(name="spool", bufs=2) as spool,
            tc.tile_pool(name="psA", bufs=6, space="PSUM") as psA,
            tc.tile_pool(name="psB", bufs=2, space="PSUM") as psB,
        ):
            # ---- resident tensors ----
            # W2 stays resident in SBUF; its slices are loaded just-in-time
            # inside chunk 0's m-loop so the first matmuls don't stall on a
            # bulk 8MB load.
            w2_sb = singles.tile([P, KH, O], f16)
            wgn_sb = singles.tile([P, KD, 2 * E], f32)
            nc.sync.dma_start(wgn_sb[:], wgn_d[:])
            bgbn_sb = singles.tile([P, 2 * E], f32)
            nc.sync.dma_start(bgbn_sb[:], bgbn_d[0].partition_broadcast(P))
            b1_sb = singles.tile([P, NM], f32)
            nc.sync.dma_start(b1_sb[:], b1_d[:])
            b2_sb = singles.tile([P, O], f32)
            sel_sb = singles.tile([P, E], f32)
            nc.sync.dma_start(sel_sb[:], sel_d[0].partition_broadcast(P))
            nz_sb = singles.tile([P, NTB, E], f32)
            nc.sync.dma_start(nz_sb[:], nz_d[:])

            for ci, (tb0, ntb) in enumerate(chunks):
                nt = ntb * P
                xs = xpool.tile([P, KD, TB_PER_CHUNK * P], f16, tag="xs")
                nc.sync.dma_start(
                    xs[:, :, :nt],
                    xh_d[:, :, tb0 * P : tb0 * P + nt],
                )

                # ---- routing for this chunk's token blocks ----
                gate = gpool.tile([P, TB_PER_CHUNK], f32, tag="gate")
                for j in range(ntb):
                    tb = tb0 + j
                    # full-fp32 x tile for routing (top-2 selection needs
                    # more precision than fp32r)
                    xr = rpool.tile([P, KD, P], f32, tag="xr")
                    nc.sync.dma_start(
                        xr[:], xT_d[:, :, (tb0 + j) * P : (tb0 + j + 1) * P]
                    )
                    ps = psB.tile([P, TB_PER_CHUNK * P], f32, tag="mm1ps")
                    pg = ps[:, : 2 * E]
                    for ko in range(KD):
                        nc.tensor.matmul(
                            pg,
                            xr[:, ko, :],
                            wgn_sb[:, ko, :],
                            start=(ko == 0),
                            stop=(ko == KD - 1),
                        )
                    nsb = rpool.tile([P, 2 * E], f32, tag="nsb")
                    nc.vector.tensor_add(nsb[:], pg, bgbn_sb[:])
                    # softplus = ln(1 + exp(z)); |z| <~ 5 here so exp is safe
                    spt = rpool.tile([P, E], f32, tag="spt")
                    nc.scalar.activation(spt[:], nsb[:, E:], AF.Exp)
                    nc.vector.tensor_scalar_add(spt[:], spt[:], 1.0)
                    nc.scalar.activation(spt[:], spt[:], AF.Ln)
                    noisy = rpool.tile([P, E], f32, tag="noisy")
                    nc.vector.tensor_mul(noisy[:], spt[:], nz_sb[:, tb, :])
                    nc.vector.tensor_add(noisy[:], noisy[:], nsb[:, :E])
                    m1 = rpool.tile([P, 1], f32, tag="m1")
                    nc.vector.reduce_max(m1[:], noisy[:], axis=AX.X)
                    eq = rpool.tile([P, E], f32, tag="eq")
                    nc.vector.tensor_scalar(
                        eq[:], noisy[:], m1[:], None, ALU.is_ge
                    )
                    nc.vector.tensor_scalar_mul(eq[:], eq[:], -1e30)
                    nc.vector.tensor_add(eq[:], eq[:], noisy[:])
                    m2 = rpool.tile([P, 1], f32, tag="m2")
                    nc.vector.reduce_max(m2[:], eq[:], axis=AX.X)
                    m1n = rpool.tile([P, 1], f32, tag="m1n")
                    nc.vector.tensor_scalar_mul(m1n[:], m1[:], -1.0)
                    ex = rpool.tile([P, E], f32, tag="ex")
                    nc.scalar.activation(ex[:], noisy[:], AF.Exp, bias=m1n[:])
                    r2 = rpool.tile([P, 1], f32, tag="r2")
                    nc.scalar.activation(r2[:], m2[:], AF.Exp, bias=m1n[:])
                    nc.vector.tensor_scalar_add(r2[:], r2[:], 1.0)
                    rden = rpool.tile([P, 1], f32, tag="rden")
                    nc.vector.reciprocal(rden[:], r2[:])
                    msk = rpool.tile([P, E], f32, tag="msk")
                    nc.vector.tensor_scalar(
                        msk[:], noisy[:], m2[:], None, ALU.is_ge
                    )
                    nc.vector.tensor_mul(msk[:], msk[:], ex[:])
                    nc.vector.tensor_scalar_mul(msk[:], msk[:], rden[:])
                    nc.vector.tensor_mul(msk[:], msk[:], sel_sb[:])
                    nc.vector.reduce_sum(gate[:, j : j + 1], msk[:], axis=AX.X)

                if ci == 0:
                    nc.sync.dma_start(b2_sb[:], b2_d[0].partition_broadcast(P))

                # ---- fused FFN: MM1 (per H-slice) -> MM2 accumulate ----
                # Software-pipelined: MM2 consumes hm[m - DELTA] while MM1
                # produces hm[m], so the PE always has independent MM1 work
                # while MM2 waits on the relu eviction / psum slots.
                DELTA = 2
                accs = [
                    [
                        psA.tile([P, 512], f32, tag="acc", name=f"acc_{j}_{osl}")
                        for osl in range(OS)
                    ]
                    for j in range(ntb)
                ]
                hms = {}
                for m in range(NM + DELTA):
                    if m < NM:
                        w1t = w1pool.tile([P, KD, P], f16, tag="w1t")
                        nc.sync.dma_start(w1t[:], w1_d[m])
                        if ci == 0:
                            nc.sync.dma_start(w2_sb[:, m, :], w2_d[:, m, :])
                        hps = psB.tile([P, TB_PER_CHUNK * P], f32, tag="mm1ps")
                        hw = hps[:, :nt]
                        for ko in range(KD):
                            nc.tensor.matmul(
                                hw,
                                w1t[:, ko, :],
                                xs[:, ko, :nt],
                                start=(ko == 0),
                                stop=(ko == KD - 1),
                            )
                        hm = hpool.tile([P, TB_PER_CHUNK * P], f16, tag="hm")
                        nc.scalar.activation(
                            hm[:, :nt], hw, AF.Relu, bias=b1_sb[:, m : m + 1]
                        )
                        hms[m] = hm
                    if m >= DELTA:
                        mm = m - DELTA
                        hm2 = hms.pop(mm)
                        for j in range(ntb):
                            for osl in range(OS):
                                nc.tensor.matmul(
                                    accs[j][osl],
                                    hm2[:, j * P : (j + 1) * P],
                                    w2_sb[:, mm, osl * 512 : (osl + 1) * 512],
                                    start=(mm == 0),
                                    stop=(mm == NM - 1),
                                )

                # ---- evict: (acc + b2) * gate -> DRAM ----
                for j in range(ntb):
                    st = spool.tile([P, O], f32, tag="st")
                    for osl in range(OS):
                        sl = slice(osl * 512, (osl + 1) * 512)
                        nc.vector.tensor_add(st[:, sl], accs[j][osl], b2_sb[:, sl])
                        nc.vector.tensor_scalar_mul(
                            st[:, sl], st[:, sl], gate[:, j : j + 1]
                        )
                    nc.sync.dma_start(out_d[tb0 + j], st[:])

    nc.finalize()
    return nc


def _routing_host(xf, nf, Wg, bg, Wn, bn):
    """Top-2 expert indices per token (selection only; gates are computed
    on device)."""
    logits = xf @ Wg + bg
    nl = xf @ Wn + bn
    sp = np.logaddexp(0.0, nl)
    noisy = logits + nf * sp
    order = np.argpartition(-noisy, 2, axis=1)[:, :2]
    mask = np.zeros(noisy.shape, dtype=bool)
    mask[np.arange(noisy.shape[0])[:, None], order] = True
    return mask


def _prep_core(xf, nf, idx, C, Wg, Wn, bg, bn, W1e, b1e, W2e, b2e, e):
    n = len(idx)
    x_g = np.zeros((C, D), np.float32)
    x_g[:n] = xf[idx]
    nz_g = np.zeros((C, E), np.float32)
    nz_g[:n] = nf[idx]
    NTB = C // P
    xT = np.ascontiguousarray(x_g.reshape(C, KD, P).transpose(2, 1, 0))
    return {
        "xT": xT,
        "xh": xT.astype(np.float16),
        "w1s": np.ascontiguousarray(
            W1e.reshape(KD, P, NM, P).transpose(2, 1, 0, 3)
        ).astype(np.float16),
        "w2s": np.ascontiguousarray(
            W2e.reshape(KH, P, O).transpose(1, 0, 2)
        ).astype(np.float16),
        "wgn": np.ascontiguousarray(
            np.concatenate([Wg, Wn], axis=1).reshape(KD, P, 2 * E).transpose(1, 0, 2)
        ),
        "bgbn": np.concatenate([bg, bn])[None, :].astype(np.float32),
        "b1s": np.ascontiguousarray(b1e.reshape(NM, P).T),
        "b2e": b2e[None, :].astype(np.float32),
        "noise_g": np.ascontiguousarray(
            nz_g.reshape(NTB, P, E).transpose(1, 0, 2)
        ),
        "sel": np.eye(E, dtype=np.float32)[e][None, :],
    }


def kernel(x, noise, Wg, bg, Wn, bn, W1, b1, W2, b2):
    from concourse.bass_utils import run_bass_kernel_spmd

    x = np.asarray(x, np.float32)
    noise = np.asarray(noise, np.float32)
    Wg = np.asarray(Wg, np.float32)
    bg = np.asarray(bg, np.float32)
    Wn = np.asarray(Wn, np.float32)
    bn = np.asarray(bn, np.float32)
    W1 = np.asarray(W1, np.float32)
    b1 = np.asarray(b1, np.float32)
    W2 = np.asarray(W2, np.float32)
    b2 = np.asarray(b2, np.float32)

    Bx, Tx, _ = x.shape
    ntok = Bx * Tx
    xf = x.reshape(ntok, D)
    nf = noise.reshape(ntok, E)

    mask = _routing_host(xf, nf, Wg, bg, Wn, bn)
    idx = [np.nonzero(mask[:, e])[0] for e in range(E)]
    C = max(P, int(np.ceil(max(len(i) for i in idx) / P) * P))

    if C not in _NC_CACHE:
        _NC_CACHE[C] = _build_nc(C)
    nc = _NC_CACHE[C]

    in_maps = [
        _prep_core(
            xf, nf, idx[e], C, Wg, Wn, bg, bn, W1[e], b1[e], W2[e], b2[e], e
        )
        for e in range(E)
    ]

    trace = bool(os.environ.get("MOE_TRACE"))
    t0 = time.time()
    res = run_bass_kernel_spmd(
        nc, in_maps, list(range(E)), trace=trace
    )
    t1 = time.time()
    LAST_RUN.clear()
    LAST_RUN.update(
        wall_s=t1 - t0,
        exec_time_ns=res.exec_time_ns,
        trace=res.instructions_and_trace[1] if res.instructions_and_trace else None,
    )

    out = np.zeros((ntok, O), np.float32)
    for e in range(E):
        n = len(idx[e])
        y = res.results[e]["out"].reshape(C, O)
        out[idx[e]] += y[:n]
    return out.reshape(Bx, Tx, O)


# revision 22
# speedup vs baseline: 1.3116x; 1.0383x over previous
"""Trainium2 Bass kernel for the NoisyTopK MoE layer (B=2,T=2048,D=1024,H=4096,O=1024,E=8,K=2).

Strategy (expert-parallel, 8 cores = 8 experts):
  * Host: compute top-2 routing indices (tiny numpy), gather each expert's
    tokens, pad to a common capacity C (multiple of 128).
  * Device (per core, SPMD — same program, per-expert data):
      - routing: noisy logits -> top-2 -> sparse softmax gates, on-chip,
        for the gathered tokens (replicated gate params)
      - expert FFN: out = (relu(x @ W1 + b1) @ W2 + b2) * gate_e
        fused MM1->MM2 per H-slice, fp32r matmuls, W2 resident in SBUF,
        W1 streamed per chunk of 384 tokens.
  * Host: scatter-add the per-expert outputs back to [B,T,O]
    (equivalent to the all-reduce of the gated combine).
"""

import os
import time

import numpy as np

P = 128
B, T, D, H, O, E = 2, 2048, 1024, 4096, 1024, 8
KD = D // P   # 8  k-tiles over D
KH = H // P   # 32 k-tiles over H (= number of m-slices of MM1)
NM = H // P   # 32 m-slices
OS = 2        # O-slices of 512
TB_PER_CHUNK = 3  # 384 tokens per chunk

_NC_CACHE = {}
LAST_RUN = {}


def _build_nc(C):
    import concourse.mybir as mybir
    import concourse.tile as tile
    from concourse import bacc

    f32 = mybir.dt.float32
    f16 = mybir.dt.float16
    AF = mybir.ActivationFunctionType
    ALU = mybir.AluOpType
    AX = mybir.AxisListType

    NTB = C // P
    chunks = []
    tb0 = 0
    while tb0 < NTB:
        n = min(TB_PER_CHUNK, NTB - tb0)
        chunks.append((tb0, n))
        tb0 += n

    # Bacc (not plain Bass): its compile() pass splits multi-wait matmuls
    # (HW allows a single sync-wait on the fused LDWEIGHTS+MATMULT).
    nc = bacc.Bacc()
    xT_d = nc.declare_dram_parameter("xT", [P, KD, C], f32, isOutput=False)
    xh_d = nc.declare_dram_parameter("xh", [P, KD, C], f16, isOutput=False)
    w1_d = nc.declare_dram_parameter("w1s", [NM, P, KD, P], f16, isOutput=False)
    w2_d = nc.declare_dram_parameter("w2s", [P, KH, O], f16, isOutput=False)
    wgn_d = nc.declare_dram_parameter("wgn", [P, KD, 2 * E], f32, isOutput=False)
    bgbn_d = nc.declare_dram_parameter("bgbn", [1, 2 * E], f32, isOutput=False)
    b1_d = nc.declare_dram_parameter("b1s", [P, NM], f32, isOutput=False)
    b2_d = nc.declare_dram_parameter("b2e", [1, O], f32, isOutput=False)
    nz_d = nc.declare_dram_parameter("noise_g", [P, NTB, E], f32, isOutput=False)
    sel_d = nc.declare_dram_parameter("sel", [1, E], f32, isOutput=False)
    out_d = nc.declare_dram_parameter("out", [NTB, P, O], f32, isOutput=True)

    with tile.TileContext(nc) as tc:
        with (
            tc.tile_pool(name="singles", bufs=1) as singles,
            tc.tile_pool(name="w1pool", bufs=3) as w1pool,
            tc.tile_pool(name="xpool", bufs=2) as xpool,
            tc.tile_pool(name="hpool", bufs=6) as hpool,
            tc.tile_pool(name="rpool", bufs=2) as rpool,
            tc.tile_pool(name="gpool", bufs=2) as gpool,
            tc.tile_pool(name="spool", bufs=2) as spool,
            tc.tile_pool(name="psA", bufs=6, space="PSUM") as psA,
            tc.tile_pool(name="psB", bufs=2, space="PSUM") as psB,
        ):
            # ---- resident tensors ----
            # W2 stays resident in SBUF; its slices are loaded just-in-time
            # inside chunk 0's m-loop so the first matmuls don't stall on a
            # bulk 8MB load.
            w2_sb = singles.tile([P, KH, O], f16)
            wgn_sb = singles.tile([P, KD, 2 * E], f32)
            nc.sync.dma_start(wgn_sb[:], wgn_d[:])
            bgbn_sb = singles.tile([P, 2 * E], f32)
            nc.sync.dma_start(bgbn_sb[:], bgbn_d[0].partition_broadcast(P))
            b1_sb = singles.tile([P, NM], f32)
            nc.sync.dma_start(b1_sb[:], b1_d[:])
            b2_sb = singles.tile([P, O], f32)
            sel_sb = singles.tile([P, E], f32)
            nc.sync.dma_start(sel_sb[:], sel_d[0].partition_broadcast(P))
            nz_sb = singles.tile([P, NTB, E], f32)
            nc.sync.dma_start(nz_sb[:], nz_d[:])

            def emit_routing(tb0, ntb, gate):
                """Noisy-top2 gates for one chunk's token blocks (fp32)."""
                for j in range(ntb):
                    tb = tb0 + j
                    # full-fp32 x tile for routing (top-2 selection needs
                    # more precision than fp16)
                    xr = rpool.tile([P, KD, P], f32, tag="xr", name="xr")
                    nc.sync.dma_start(
                        xr[:], xT_d[:, :, tb * P : (tb + 1) * P]
                    )
                    ps = psB.tile(
                        [P, TB_PER_CHUNK * P], f32, tag="mm1ps", name="rps"
                    )
                    pg = ps[:, : 2 * E]
                    for ko in range(KD):
                        nc.tensor.matmul(
                            pg,
                            xr[:, ko, :],
                            wgn_sb[:, ko, :],
                            start=(ko == 0),
                            stop=(ko == KD - 1),
                        )
                    nsb = rpool.tile([P, 2 * E], f32, tag="nsb", name="nsb")
                    nc.vector.tensor_add(nsb[:], pg, bgbn_sb[:])
                    # softplus = ln(1 + exp(z)); |z| <~ 5 here so exp is safe
                    spt = rpool.tile([P, E], f32, tag="spt", name="spt")
                    nc.scalar.activation(spt[:], nsb[:, E:], AF.Exp)
                    nc.vector.tensor_scalar_add(spt[:], spt[:], 1.0)
                    nc.scalar.activation(spt[:], spt[:], AF.Ln)
                    noisy = rpool.tile([P, E], f32, tag="noisy", name="noisy")
                    nc.vector.tensor_mul(noisy[:], spt[:], nz_sb[:, tb, :])
                    nc.vector.tensor_add(noisy[:], noisy[:], nsb[:, :E])
                    m1 = rpool.tile([P, 1], f32, tag="m1", name="m1")
                    nc.vector.reduce_max(m1[:], noisy[:], axis=AX.X)
                    eq = rpool.tile([P, E], f32, tag="eq", name="eq")
                    nc.vector.tensor_scalar(
                        eq[:], noisy[:], m1[:], None, ALU.is_ge
                    )
                    nc.vector.tensor_scalar_mul(eq[:], eq[:], -1e30)
                    nc.vector.tensor_add(eq[:], eq[:], noisy[:])
                    m2 = rpool.tile([P, 1], f32, tag="m2", name="m2")
                    nc.vector.reduce_max(m2[:], eq[:], axis=AX.X)
                    m1n = rpool.tile([P, 1], f32, tag="m1n", name="m1n")
                    nc.vector.tensor_scalar_mul(m1n[:], m1[:], -1.0)
                    ex = rpool.tile([P, E], f32, tag="ex", name="ex")
                    nc.scalar.activation(ex[:], noisy[:], AF.Exp, bias=m1n[:])
                    r2 = rpool.tile([P, 1], f32, tag="r2", name="r2")
                    nc.scalar.activation(r2[:], m2[:], AF.Exp, bias=m1n[:])
                    nc.vector.tensor_scalar_add(r2[:], r2[:], 1.0)
                    rden = rpool.tile([P, 1], f32, tag="rden", name="rden")
                    nc.vector.reciprocal(rden[:], r2[:])
                    msk = rpool.tile([P, E], f32, tag="msk", name="msk")
                    nc.vector.tensor_scalar(
                        msk[:], noisy[:], m2[:], None, ALU.is_ge
                    )
                    nc.vector.tensor_mul(msk[:], msk[:], ex[:])
                    nc.vector.tensor_scalar_mul(msk[:], msk[:], rden[:])
                    nc.vector.tensor_mul(msk[:], msk[:], sel_sb[:])
                    nc.vector.reduce_sum(gate[:, j : j + 1], msk[:], axis=AX.X)

            # routing for chunk 0 runs first: its small DMAs land quickly, so
            # the PE starts within ~2us while the bulk x/weight loads stream.
            gates = {0: gpool.tile([P, TB_PER_CHUNK], f32, tag="gate", name="g0")}
            emit_routing(chunks[0][0], chunks[0][1], gates[0])

            # MM2 trails MM1 by DELTA H-slices: the PE always has independent
            # MM1 work while MM2 waits on relu eviction / psum-slot release.
            DELTA = 4

            for ci, (tb0, ntb) in enumerate(chunks):
                nt = ntb * P
                xs = xpool.tile([P, KD, TB_PER_CHUNK * P], f16, tag="xs")
                for ko in range(KD):  # split across DMA queues
                    nc.sync.dma_start(
                        xs[:, ko, :nt],
                        xh_d[:, ko, tb0 * P : tb0 * P + nt],
                    )
                gate = gates.pop(ci)
                if ci == 0:
                    nc.sync.dma_start(b2_sb[:], b2_d[0].partition_broadcast(P))
                accs = [
                    [
                        psA.tile([P, 512], f32, tag="acc", name=f"acc_{j}_{osl}")
                        for osl in range(OS)
                    ]
                    for j in range(ntb)
                ]
                hms = {}
                for m in range(NM + DELTA):
                    if m == 10 and ci + 1 < len(chunks):
                        # prefetch next chunk's routing while this chunk's
                        # m-loop keeps the PE saturated
                        gates[ci + 1] = gpool.tile(
                            [P, TB_PER_CHUNK], f32, tag="gate", name="g"
                        )
                        emit_routing(
                            chunks[ci + 1][0], chunks[ci + 1][1], gates[ci + 1]
                        )
                    if m < NM:
                        w1t = w1pool.tile([P, KD, P], f16, tag="w1t")
                        nc.sync.dma_start(w1t[:], w1_d[m])
                        if ci == 0:
                            nc.sync.dma_start(w2_sb[:, m, :], w2_d[:, m, :])
                        hps = psB.tile([P, TB_PER_CHUNK * P], f32, tag="mm1ps")
                        hw = hps[:, :nt]
                        for ko in range(KD):
                            nc.tensor.matmul(
                                hw,
                                w1t[:, ko, :],
                                xs[:, ko, :nt],
                                start=(ko == 0),
                                stop=(ko == KD - 1),
                            )
                        hm = hpool.tile([P, TB_PER_CHUNK * P], f16, tag="hm")
                        nc.scalar.activation(
                            hm[:, :nt], hw, AF.Relu, bias=b1_sb[:, m : m + 1]
                        )
                        hms[m] = hm
                    if m >= DELTA:
                        mm = m - DELTA
                        hm2 = hms.pop(mm)
                        for j in range(ntb):
                            for osl in range(OS):
                                nc.tensor.matmul(
                                    accs[j][osl],
                                    hm2[:, j * P : (j + 1) * P],
                                    w2_sb[:, mm, osl * 512 : (osl + 1) * 512],
                                    start=(mm == 0),
                                    stop=(mm == NM - 1),
                                )

                # ---- evict: (acc + b2) * gate -> DRAM ----
                for j in range(ntb):
                    st = spool.tile([P, O], f32, tag="st")
                    for osl in range(OS):
                        sl = slice(osl * 512, (osl + 1) * 512)
                        nc.vector.tensor_add(st[:, sl], accs[j][osl], b2_sb[:, sl])
                        nc.vector.tensor_scalar_mul(
                            st[:, sl], st[:, sl], gate[:, j : j + 1]
                        )
                    nc.sync.dma_start(out_d[tb0 + j], st[:])

    nc.finalize()
    return nc


def _routing_host(xf, nf, Wg, bg, Wn, bn):
    """Top-2 expert indices per token (selection only; gates are computed
    on device)."""
    logits = xf @ Wg + bg
    nl = xf @ Wn + bn
    sp = np.logaddexp(0.0, nl)
    noisy = logits + nf * sp
    order = np.argpartition(-noisy, 2, axis=1)[:, :2]
    mask = np.zeros(noisy.shape, dtype=bool)
    mask[np.arange(noisy.shape[0])[:, None], order] = True
    return mask


def _prep_core(xf, nf, idx, C, Wg, Wn, bg, bn, W1e, b1e, W2e, b2e, e):
    n = len(idx)
    x_g = np.zeros((C, D), np.float32)
    x_g[:n] = xf[idx]
    nz_g = np.zeros((C, E), np.float32)
    nz_g[:n] = nf[idx]
    NTB = C // P
    xT = np.ascontiguousarray(x_g.reshape(C, KD, P).transpose(2, 1, 0))
    return {
        "xT": xT,
        "xh": xT.astype(np.float16),
        "w1s": np.ascontiguousarray(
            W1e.reshape(KD, P, NM, P).transpose(2, 1, 0, 3)
        ).astype(np.float16),
        "w2s": np.ascontiguousarray(
            W2e.reshape(KH, P, O).transpose(1, 0, 2)
        ).astype(np.float16),
        "wgn": np.ascontiguousarray(
            np.concatenate([Wg, Wn], axis=1).reshape(KD, P, 2 * E).transpose(1, 0, 2)
        ),
        "bgbn": np.concatenate([bg, bn])[None, :].astype(np.float32),
        "b1s": np.ascontiguousarray(b1e.reshape(NM, P).T),
        "b2e": b2e[None, :].astype(np.float32),
        "noise_g": np.ascontiguousarray(
            nz_g.reshape(NTB, P, E).transpose(1, 0, 2)
        ),
        "sel": np.eye(E, dtype=np.float32)[e][None, :],
    }


def kernel(x, noise, Wg, bg, Wn, bn, W1, b1, W2, b2):
    from concourse.bass_utils import run_bass_kernel_spmd

    x = np.asarray(x, np.float32)
    noise = np.asarray(noise, np.float32)
    Wg = np.asarray(Wg, np.float32)
    bg = np.asarray(bg, np.float32)
    Wn = np.asarray(Wn, np.float32)
    bn = np.asarray(bn, np.float32)
    W1 = np.asarray(W1, np.float32)
    b1 = np.asarray(b1, np.float32)
    W2 = np.asarray(W2, np.float32)
    b2 = np.asarray(b2, np.float32)

    Bx, Tx, _ = x.shape
    ntok = Bx * Tx
    xf = x.reshape(ntok, D)
    nf = noise.reshape(ntok, E)

    mask = _routing_host(xf, nf, Wg, bg, Wn, bn)
    idx = [np.nonzero(mask[:, e])[0] for e in range(E)]
    C = max(P, int(np.ceil(max(len(i) for i in idx) / P) * P))

    if C not in _NC_CACHE:
        _NC_CACHE[C] = _build_nc(C)
    nc = _NC_CACHE[C]

    in_maps = [
        _prep_core(
            xf, nf, idx[e], C, Wg, Wn, bg, bn, W1[e], b1[e], W2[e], b2[e], e
        )
        for e in range(E)
    ]

    trace = bool(os.environ.get("MOE_TRACE"))
    t0 = time.time()
    res = run_bass_kernel_spmd(
        nc, in_maps, list(range(E)), trace=trace
    )
    t1 = time.time()
    LAST_RUN.clear()
    LAST_RUN.update(
        wall_s=t1 - t0,
        exec_time_ns=res.exec_time_ns,
        trace=res.instructions_and_trace[1] if res.instructions_and_trace else None,
    )

    out = np.zeros((ntok, O), np.float32)
    for e in range(E):
        n = len(idx[e])
        y = res.results[e]["out"].reshape(C, O)
        out[idx[e]] += y[:n]
    return out.reshape(Bx, Tx, O)
